# Initial kernel scaffold
#
"""Trainium2 Bass kernel for sliding-window GQA attention prefill.

Model (hardcoded from the problem spec):
  x:[2048,4096] f32, 32 q heads / 8 kv heads, head_dim 128, rope interleaved,
  causal mask (sliding window 4096 >= seqlen 2048 so pure causal),
  positions == arange(2048)  (prefill: cache rows [0,2048) replaced).

Distribution: tensor-parallel over heads across 8 NeuronCores.
Core c owns q heads [4c,4c+4) and kv head c:
  - wq/wk/wv output rows sharded, wo input cols sharded, x replicated.
  - attention computed fully locally (GQA group is core-local),
  - wo partial products ReduceScattered over the model dim, transposed
    on-device, concatenated on host.

Layout trick: everything lives transposed ([feature, seq]) so every matmul
is a natural PE op (contraction on partitions).  RoPE's interleaved channel
pairs are handled by permuting wq/wk rows host-side to [evens..., odds...],
making RoPE 3 full-width vector ops; cache_k is un-permuted on the way out
with strided copies after a PE transpose.
"""

import os
import numpy as np

import concourse.bass as bass
import concourse.bacc as bacc
import concourse.mybir as mybir
import concourse.tile as tile
from concourse.bass_utils import run_bass_kernel_spmd
from concourse.masks import make_identity

N_CORES = 8
S = 2048
D = 4096
HD = 128
NQ = 4  # q heads per core
P = 128
SCH = 512  # seq chunk (matmul moving free dim)
NCH = S // SCH  # 4
NKD = D // P  # 32 contraction tiles for qkv
SCALE = HD ** -0.5
NEG_CLAMP = -100.0  # exp(-100 + few) == 0 in f32 for our score range

F32 = mybir.dt.float32
F32R = mybir.dt.float32r


def _r(ap):
    return ap.bitcast(F32R)


def build_body(nc, tc, single_core=False):
    """Emit the per-core kernel body inside TileContext tc."""
    ExitC = __import__("contextlib").ExitStack()

    xT = nc.dram_tensor("xT", [D, S], F32, kind="ExternalInput").ap()
    wqT = nc.dram_tensor("wqT", [D, NQ * HD], F32, kind="ExternalInput").ap()
    wkT = nc.dram_tensor("wkT", [D, HD], F32, kind="ExternalInput").ap()
    wvT = nc.dram_tensor("wvT", [D, HD], F32, kind="ExternalInput").ap()
    woT = nc.dram_tensor("woT", [NQ * HD, D], F32, kind="ExternalInput").ap()
    cc_d = nc.dram_tensor("cc", [P, S], F32, kind="ExternalInput").ap()
    ss_d = nc.dram_tensor("ss", [P, S], F32, kind="ExternalInput").ap()
    tri_d = nc.dram_tensor("tri", [P, P], F32, kind="ExternalInput").ap()
    ck_in = nc.dram_tensor("cache_k_in", [2 * S, HD], F32, kind="ExternalInput").ap()
    cv_in = nc.dram_tensor("cache_v_in", [2 * S, HD], F32, kind="ExternalInput").ap()

    y_ext = nc.dram_tensor("y", [S, NQ * HD], F32, kind="ExternalOutput").ap()
    ck_ext = nc.dram_tensor("cache_k", [2 * S, HD], F32, kind="ExternalOutput").ap()
    cv_ext = nc.dram_tensor("cache_v", [2 * S, HD], F32, kind="ExternalOutput").ap()

    xT3 = xT.rearrange("(ko p) s -> p ko s", p=P)       # [128, 32, 2048]
    wqT3 = wqT.rearrange("(ko p) o -> p ko o", p=P)     # [128, 32, 512]
    wkT3 = wkT.rearrange("(ko p) o -> p ko o", p=P)     # [128, 32, 128]
    wvT3 = wvT.rearrange("(ko p) o -> p ko o", p=P)
    woT3 = woT.rearrange("(ko p) e -> p ko e", p=P)     # [128, 4, 4096]

    with (
        tc.tile_pool(name="const", bufs=1) as cpool,
        tc.tile_pool(name="wres", bufs=1) as wres,
        tc.tile_pool(name="res", bufs=1) as res,
        tc.tile_pool(name="stream", bufs=3) as stream,
        tc.tile_pool(name="qres", bufs=2) as qres,
        tc.tile_pool(name="rope", bufs=2) as rope,
        tc.tile_pool(name="expp", bufs=4) as expp,
        tc.tile_pool(name="outp", bufs=3) as outp,
        tc.tile_pool(name="psA", bufs=1, space="PSUM") as psA,
        tc.tile_pool(name="psB", bufs=2, space="PSUM") as psB,
        tc.tile_pool(name="dram", bufs=1, space="DRAM") as dram,
    ):
        # ---- constants ----
        ident = cpool.tile([P, P], F32, tag="ident")
        make_identity(nc, ident[:])
        ones = cpool.tile([P, 1], F32, tag="ones")
        nc.vector.memset(ones[:], 1.0)
        cc = cpool.tile([P, S], F32, tag="cc")
        nc.sync.dma_start(cc[:], cc_d[:])
        ss = cpool.tile([P, S], F32, tag="ss")
        nc.sync.dma_start(ss[:], ss_d[:])
        tri = cpool.tile([P, P], F32, tag="tri")
        nc.sync.dma_start(tri[:], tri_d[:])

        # ---- resident tensors ----
        # wq resident (8MB), split DMAs per k-tile so matmuls start early
        wq_sb = wres.tile([P, NKD, NQ * HD], F32, tag="wq")
        for kd in range(NKD):
            nc.sync.dma_start(wq_sb[:, kd, :], wqT3[:, kd, :])
        kT = res.tile([P, S], F32, tag="kT")          # rope'd k, permuted chans
        v_sb = res.tile([P, S // P, HD], F32, tag="v")  # v rows [j within tile, jt, d]
        qT = {}   # (h, c) -> [128, 512] tile, allocated per chunk
        outT = {}  # (h, c) -> [128, 512]

        # cache passthrough rows [2048, 4096)
        nc.sync.dma_start(ck_ext[S:, :], ck_in[S:, :])
        nc.sync.dma_start(cv_ext[S:, :], cv_in[S:, :])

        yT_dram = dram.tile([D, S], F32, tag="yT")
        yT_rs = dram.tile([D // N_CORES, S], F32, tag="yT_rs")

        def rope_evict(psum, dst, sl):
            """dst[:, :] = psum * cc[:, sl] + swap(psum) * ss[:, sl]."""
            tmp = rope.tile([P, SCH], F32, tag="swap")
            nc.vector.tensor_copy(out=tmp[0:64, :], in_=psum[64:128, :])
            nc.vector.tensor_copy(out=tmp[64:128, :], in_=psum[0:64, :])
            t1 = rope.tile([P, SCH], F32, tag="ropet1")
            nc.vector.tensor_mul(out=t1[:], in0=psum[:], in1=cc[:, sl])
            nc.vector.tensor_mul(out=tmp[:], in0=tmp[:], in1=ss[:, sl])
            nc.vector.tensor_add(out=dst, in0=t1[:], in1=tmp[:])

        for c in range(NCH):
            sl = slice(c * SCH, (c + 1) * SCH)
            # ================= QKV for chunk c =================
            pq = [psA.tile([P, SCH], F32, tag=f"pq{h}") for h in range(NQ)]
            pk = psA.tile([P, SCH], F32, tag="pk")
            pv = psA.tile([P, SCH], F32, tag="pv")
            for kd in range(NKD):
                xt = stream.tile([P, SCH], F32, tag="xt")
                nc.sync.dma_start(xt[:], xT3[:, kd, sl])
                wk_t = stream.tile([P, HD], F32, tag="wk")
                nc.sync.dma_start(wk_t[:], wkT3[:, kd, :])
                wv_t = stream.tile([P, HD], F32, tag="wv")
                nc.sync.dma_start(wv_t[:], wvT3[:, kd, :])
                st = kd == 0
                sp = kd == NKD - 1
                for h in range(NQ):
                    nc.tensor.matmul(
                        pq[h][:], lhsT=_r(wq_sb[:, kd, h * HD:(h + 1) * HD]),
                        rhs=_r(xt[:]), start=st, stop=sp)
                nc.tensor.matmul(pk[:], lhsT=_r(wk_t[:]), rhs=_r(xt[:]),
                                 start=st, stop=sp)
                nc.tensor.matmul(pv[:], lhsT=_r(wv_t[:]), rhs=_r(xt[:]),
                                 start=st, stop=sp)

            # rope q -> per-chunk qT tiles
            for h in range(NQ):
                qt = qres.tile([P, SCH], F32, tag=f"qT{h}")
                rope_evict(pq[h][:], qt[:], sl)
                qT[(h, c)] = qt
            # rope k -> resident kT
            rope_evict(pk[:], kT[:, sl], sl)

            # v: psum -> sbuf, then transpose into v_sb rows; also cache_v out
            vstage = rope.tile([P, SCH], F32, tag="vstage")
            nc.scalar.copy(out=vstage[:], in_=pv[:])
            for st_i in range(SCH // P):
                jt = c * (SCH // P) + st_i
                pt = psB.tile([P, P], F32, tag="tp")
                nc.tensor.transpose(pt[:], vstage[:, st_i * P:(st_i + 1) * P], ident[:])
                nc.any.tensor_copy(out=v_sb[:, jt, :], in_=pt[:])
                nc.sync.dma_start(cv_ext[jt * P:(jt + 1) * P, :], v_sb[:, jt, :])

            # cache_k out: transpose kT chunk, un-permute channels (stride-2)
            for st_i in range(SCH // P):
                jt = c * (SCH // P) + st_i
                pt = psB.tile([P, P], F32, tag="tp")
                nc.tensor.transpose(pt[:], kT[:, jt * P:(jt + 1) * P], ident[:])
                ck_t = rope.tile([P, P], F32, tag="ck")
                nc.vector.tensor_copy(out=ck_t[:, 0:P:2], in_=pt[:, 0:64])
                nc.vector.tensor_copy(out=ck_t[:, 1:P:2], in_=pt[:, 64:128])
                nc.sync.dma_start(ck_ext[jt * P:(jt + 1) * P, :], ck_t[:])

            # ================= attention for chunk c =================
            njt = 4 * (c + 1)
            for h in range(NQ):
                pvp = psA.tile([P, SCH], F32, tag="pvp")
                rsp = psA.tile([1, SCH], F32, tag="rsp")
                qr = _r(qT[(h, c)][:])
                for ti in range(njt):
                    delta = P * ti - SCH * c
                    scp = psB.tile([P, SCH], F32, tag="sc")
                    nc.tensor.matmul(scp[:], lhsT=_r(kT[:, ti * P:(ti + 1) * P]),
                                     rhs=qr, start=True, stop=True)
                    ex = expp.tile([P, SCH], F32, tag="ex")
                    if delta > -P:  # diagonal tile
                        nc.vector.tensor_add(out=scp[:, delta:delta + P],
                                             in0=scp[:, delta:delta + P], in1=tri[:])
                        if delta > 0:
                            nc.vector.memset(ex[:, 0:delta], 0.0)
                        nc.scalar.activation(out=ex[:, delta:], in_=scp[:, delta:],
                                             func=mybir.ActivationFunctionType.Exp,
                                             scale=SCALE)
                    else:
                        nc.scalar.activation(out=ex[:], in_=scp[:],
                                             func=mybir.ActivationFunctionType.Exp,
                                             scale=SCALE)
                    nc.tensor.matmul(pvp[:], lhsT=_r(v_sb[:, ti, :]), rhs=_r(ex[:]),
                                     start=(ti == 0), stop=(ti == njt - 1))
                    nc.tensor.matmul(rsp[:], lhsT=_r(ones[:]), rhs=_r(ex[:]),
                                     start=(ti == 0), stop=(ti == njt - 1))
                rec = rope.tile([1, SCH], F32, tag="rec")
                nc.vector.reciprocal(out=rec[:], in_=rsp[:])
                bc = rope.tile([P, SCH], F32, tag="bc")
                nc.gpsimd.partition_broadcast(bc[:], rec[:], channels=P)
                ot = outp.tile([P, SCH], F32, tag=f"oT{h}")
                nc.vector.tensor_mul(out=ot[:], in0=pvp[:], in1=bc[:])
                outT[(h, c)] = ot

            # ================= wo partial for chunk c =================
            for et in range(D // P):
                wo_t = stream.tile([P, NQ, P], F32, tag="wo")
                nc.sync.dma_start(wo_t[:], woT3[:, :, et * P:(et + 1) * P])
                yp = psB.tile([P, SCH], F32, tag="yp")
                for h in range(NQ):
                    nc.tensor.matmul(yp[:], lhsT=_r(wo_t[:, h, :]),
                                     rhs=_r(outT[(h, c)][:]),
                                     start=(h == 0), stop=(h == NQ - 1))
                ysb = outp.tile([P, SCH], F32, tag="ysb")
                nc.any.tensor_copy(out=ysb[:], in_=yp[:])
                nc.sync.dma_start(yT_dram[et * P:(et + 1) * P, sl], ysb[:])

        # ================= reduce-scatter + final transpose =================
        if single_core:
            nc.sync.dma_start(yT_rs[:], yT_dram[0:D // N_CORES, :])
        else:
            nc.gpsimd.collective_compute(
                "ReduceScatter", mybir.AluOpType.add,
                replica_groups=[list(range(N_CORES))],
                ins=[yT_dram[:]], outs=[yT_rs[:]])

        for et in range(D // N_CORES // P):  # 4
            yr = outp.tile([P, S], F32, tag="yr")
            nc.sync.dma_start(yr[:], yT_rs[et * P:(et + 1) * P, :])
            for st_i in range(S // P):  # 16
                pt = psB.tile([P, P], F32, tag="tp")
                nc.tensor.transpose(pt[:], yr[:, st_i * P:(st_i + 1) * P], ident[:])
                yo = outp.tile([P, P], F32, tag="yo")
                nc.any.tensor_copy(out=yo[:], in_=pt[:])
                nc.sync.dma_start(
                    y_ext[st_i * P:(st_i + 1) * P, et * P:(et + 1) * P], yo[:])


def build_nc(single_core=False, loop_n=0):
    nc = bacc.Bacc("TRN2", target_bir_lowering=False, debug=False,
                   enable_asserts=True,
                   num_devices=1 if single_core else N_CORES)
    with tile.TileContext(nc) as tc:
        if loop_n:
            with tc.For_i(0, loop_n, 1):
                build_body(nc, tc, single_core=single_core)
        else:
            build_body(nc, tc, single_core=single_core)
    nc.compile()
    return nc


def prep_inputs(x, cos_freq, sin_freq, positions, mask, cache_k, cache_v,
                wq, wk, wv, wo):
    """Shard + pre-transpose host-side. Returns in_maps (list of 8 dicts)."""
    x = np.asarray(x, np.float32)
    cos_freq = np.asarray(cos_freq, np.float32)
    sin_freq = np.asarray(sin_freq, np.float32)
    mask = np.asarray(mask, np.float32)
    cache_k = np.asarray(cache_k, np.float32)
    cache_v = np.asarray(cache_v, np.float32)
    wq = np.asarray(wq, np.float32)
    wk = np.asarray(wk, np.float32)
    wv = np.asarray(wv, np.float32)
    wo = np.asarray(wo, np.float32)

    perm = np.concatenate([np.arange(0, HD, 2), np.arange(1, HD, 2)])
    xT = np.ascontiguousarray(x.T)
    cosT = np.ascontiguousarray(cos_freq.T)  # [64, S]
    sinT = np.ascontiguousarray(sin_freq.T)
    cc = np.concatenate([cosT, cosT], axis=0)          # [128, S]
    ss = np.concatenate([-sinT, sinT], axis=0)
    # triangle mask pattern from actual mask values, pre-divided by SCALE
    tri = np.maximum(mask[:P, :P].T, NEG_CLAMP) / SCALE
    tri = np.ascontiguousarray(tri, np.float32)

    wqh = wq.reshape(N_CORES * NQ, HD, D)[:, perm, :]
    wkh = wk.reshape(N_CORES, HD, D)[:, perm, :]
    wvh = wv.reshape(N_CORES, HD, D)

    in_maps = []
    for c in range(N_CORES):
        wq_c = wqh[NQ * c:NQ * (c + 1)].reshape(NQ * HD, D)
        in_maps.append({
            "xT": xT,
            "wqT": np.ascontiguousarray(wq_c.T),
            "wkT": np.ascontiguousarray(wkh[c].T),
            "wvT": np.ascontiguousarray(wvh[c].T),
            "woT": np.ascontiguousarray(wo[:, NQ * HD * c:NQ * HD * (c + 1)].T),
            "cc": cc,
            "ss": ss,
            "tri": tri,
            "cache_k_in": np.ascontiguousarray(cache_k[:, c, :]),
            "cache_v_in": np.ascontiguousarray(cache_v[:, c, :]),
        })
    return in_maps


_NC_CACHE = {}


def _get_nc(key=("full",), **kw):
    if key not in _NC_CACHE:
        _NC_CACHE[key] = build_nc(**kw)
    return _NC_CACHE[key]


def kernel(**inputs):
    in_maps = prep_inputs(**inputs)
    nc = _get_nc()
    res = run_bass_kernel_spmd(nc, in_maps, core_ids=list(range(N_CORES)))
    outs = res.results
    y = np.concatenate([outs[c]["y"] for c in range(N_CORES)], axis=1)
    ck = np.stack([outs[c]["cache_k"] for c in range(N_CORES)], axis=1)
    cv = np.stack([outs[c]["cache_v"] for c in range(N_CORES)], axis=1)
    return y, ck, cv


# revision 6
# speedup vs baseline: 1.7636x; 1.7636x over previous
"""Trainium2 Bass kernel for sliding-window GQA attention prefill.

Model (hardcoded from the problem spec):
  x:[2048,4096] f32, 32 q heads / 8 kv heads, head_dim 128, rope interleaved,
  causal mask (sliding window 4096 >= seqlen 2048 so pure causal),
  positions == arange(2048)  (prefill: cache rows [0,2048) replaced).

Distribution: tensor-parallel over heads across 8 NeuronCores.
Core c owns q heads [4c,4c+4) and kv head c:
  - wq/wk/wv output rows sharded, wo input cols sharded, x replicated.
  - attention computed fully locally (GQA group is core-local),
  - wo partial products ReduceScattered over the model dim, transposed
    on-device, concatenated on host.

Layout trick: everything lives transposed ([feature, seq]) so every matmul
is a natural PE op (contraction on partitions).  RoPE's interleaved channel
pairs are handled by permuting wq/wk rows host-side to [evens..., odds...],
making RoPE 3 full-width vector ops; cache_k is un-permuted on the way out
with strided copies after a PE transpose.

PSUM budget (8 banks) is shared across phases with fixed tags:
  t0..t3: qkv accum pq0..pq3 / attn pvp+rsp
  t4,t5: pk, pv / wo yp (alternating)
  t6 (2 slots): attention scores / PE-transpose staging
"""

import numpy as np

import concourse.bass as bass
import concourse.bacc as bacc
import concourse.mybir as mybir
import concourse.tile as tile
from concourse.bass_utils import run_bass_kernel_spmd
from concourse.masks import make_identity

N_CORES = 8
S = 2048
D = 4096
HD = 128
NQ = 4  # q heads per core
P = 128
SCH = 512  # seq chunk (matmul moving free dim)
NCH = S // SCH  # 4
NKD = D // P  # 32 contraction tiles for qkv
SCALE = HD ** -0.5
NEG_CLAMP = -100.0  # exp(-100 + few) == 0 in f32 for our score range

F32 = mybir.dt.float32
F32R = mybir.dt.float32r


def _r(ap):
    return ap.bitcast(F32R)


def declare_io(nc):
    io = {}
    io["xT"] = nc.dram_tensor("xT", [D, S], F32R, kind="ExternalInput").ap()
    io["wqT"] = nc.dram_tensor("wqT", [D, NQ * HD], F32R, kind="ExternalInput").ap()
    io["wkvT"] = nc.dram_tensor("wkvT", [D, 2 * HD], F32R, kind="ExternalInput").ap()
    io["woT"] = nc.dram_tensor("woT", [NQ * HD, D], F32R, kind="ExternalInput").ap()
    io["cc"] = nc.dram_tensor("cc", [P, S], F32, kind="ExternalInput").ap()
    io["ss"] = nc.dram_tensor("ss", [P, S], F32, kind="ExternalInput").ap()
    io["tri"] = nc.dram_tensor("tri", [P, P], F32, kind="ExternalInput").ap()
    io["cache_k_in"] = nc.dram_tensor("cache_k_in", [2 * S, HD], F32,
                                      kind="ExternalInput").ap()
    io["cache_v_in"] = nc.dram_tensor("cache_v_in", [2 * S, HD], F32,
                                      kind="ExternalInput").ap()
    io["y"] = nc.dram_tensor("y", [S, NQ * HD], F32, kind="ExternalOutput").ap()
    io["cache_k"] = nc.dram_tensor("cache_k", [2 * S, HD], F32,
                                   kind="ExternalOutput").ap()
    io["cache_v"] = nc.dram_tensor("cache_v", [2 * S, HD], F32,
                                   kind="ExternalOutput").ap()
    return io


def build_body(nc, tc, io, single_core=False):
    """Emit the per-core kernel body inside TileContext tc."""
    xT = io["xT"]
    wqT = io["wqT"]
    wkvT = io["wkvT"]
    woT = io["woT"]
    cc_d = io["cc"]
    ss_d = io["ss"]
    tri_d = io["tri"]
    ck_in = io["cache_k_in"]
    cv_in = io["cache_v_in"]
    y_ext = io["y"]
    ck_ext = io["cache_k"]
    cv_ext = io["cache_v"]

    xT3 = xT.rearrange("(ko p) s -> p ko s", p=P)        # [128, 32, 2048]
    wqT3 = wqT.rearrange("(ko p) o -> p ko o", p=P)      # [128, 32, 512]
    wkvT3 = wkvT.rearrange("(ko p) o -> p ko o", p=P)    # [128, 32, 256]
    woT3 = woT.rearrange("(ko p) e -> p ko e", p=P)      # [128, 4, 4096]
    ck3 = ck_ext.rearrange("(jt p) d -> p jt d", p=P)    # [128, 32, 128]
    cv3 = cv_ext.rearrange("(jt p) d -> p jt d", p=P)
    yrs3 = None  # set below

    with (
        tc.tile_pool(name="const", bufs=1) as cpool,
        tc.tile_pool(name="wres", bufs=1) as wres,
        tc.tile_pool(name="res", bufs=1) as res,
        tc.tile_pool(name="stream", bufs=3) as stream,
        tc.tile_pool(name="ccs", bufs=2) as ccs,
        tc.tile_pool(name="qres", bufs=2) as qres,
        tc.tile_pool(name="rope", bufs=2) as rope,
        tc.tile_pool(name="expp", bufs=3) as expp,
        tc.tile_pool(name="outp", bufs=2) as outp,
        tc.tile_pool(name="yout", bufs=2) as yout,
        tc.tile_pool(name="ps", bufs=1, space="PSUM") as ps,
        tc.tile_pool(name="dram", bufs=1, space="DRAM") as dram,
    ):
        # ---- constants ----
        ident = cpool.tile([P, P], F32, tag="ident", name="ident")
        make_identity(nc, ident[:])
        ones = cpool.tile([P, 1], F32R, tag="ones", name="ones")
        nc.vector.memset(ones[:].bitcast(F32), 1.0)
        tri = cpool.tile([P, P], F32, tag="tri", name="tri")
        nc.sync.dma_start(tri[:], tri_d[:])

        # ---- resident tensors ----
        # wq resident (8MB), split DMAs per k-tile so matmuls start early
        wq_sb = wres.tile([P, NKD, NQ * HD], F32R, tag="wq", name="wq")
        for kd in range(NKD):
            nc.sync.dma_start(wq_sb[:, kd, :], wqT3[:, kd, :])
        kT = res.tile([P, S], F32R, tag="kT", name="kT")          # rope'd k (permuted)
        v_sb = res.tile([P, S // P, HD], F32R, tag="v", name="v")  # [j-in-tile, jt, d]
        qT = {}   # (h, c) -> [128, 512] tile
        outT = {}  # (h, c) -> [128, 512]

        # cache passthrough rows [2048, 4096)
        nc.sync.dma_start(ck_ext[S:, :], ck_in[S:, :])
        nc.sync.dma_start(cv_ext[S:, :], cv_in[S:, :])

        yT_dram = dram.tile([D, S], F32, tag="yT", name="yT")
        yT_rs = dram.tile([D // N_CORES, S], F32, tag="yT_rs", name="yT_rs")
        yrs3 = yT_rs.rearrange("(et p) s -> p et s", p=P)    # [128, 4, 2048]

        def rope_evict(psum, dst, cct, sst):
            """dst = psum * cc + swap(psum) * ss (permuted-channel rope)."""
            tmp = rope.tile([P, SCH], F32, tag="swap", name="swap")
            nc.vector.tensor_copy(out=tmp[0:64, :], in_=psum[64:128, :])
            nc.vector.tensor_copy(out=tmp[64:128, :], in_=psum[0:64, :])
            t1 = rope.tile([P, SCH], F32, tag="ropet1", name="ropet1")
            nc.vector.tensor_mul(out=t1[:], in0=psum[:], in1=cct[:])
            nc.vector.tensor_mul(out=tmp[:], in0=tmp[:], in1=sst[:])
            nc.vector.tensor_add(out=dst, in0=t1[:], in1=tmp[:])

        def emit_attn(c):
            njt = 4 * (c + 1)
            for h in range(NQ):
                pvp = ps.tile([P, SCH], F32, tag="t0", name="pvp")
                rsp = ps.tile([1, SCH], F32, tag="t1", name="rsp")
                qr = qT[(h, c)][:]
                for ti in range(njt):
                    delta = P * ti - SCH * c
                    scp = ps.tile([P, SCH], F32, tag="t6", bufs=2, name="scp")
                    nc.tensor.matmul(scp[:], lhsT=kT[:, ti * P:(ti + 1) * P],
                                     rhs=qr, start=True, stop=True)
                    ex = expp.tile([P, SCH], F32R, tag="ex", name="ex")
                    if delta > -P:  # diagonal tile
                        nc.vector.tensor_add(out=scp[:, delta:delta + P],
                                             in0=scp[:, delta:delta + P],
                                             in1=tri[:])
                        if delta > 0:
                            nc.vector.memset(ex[:, 0:delta].bitcast(F32), 0.0)
                        nc.scalar.activation(out=ex[:, delta:], in_=scp[:, delta:],
                                             func=mybir.ActivationFunctionType.Exp,
                                             scale=SCALE)
                    else:
                        nc.scalar.activation(out=ex[:], in_=scp[:],
                                             func=mybir.ActivationFunctionType.Exp,
                                             scale=SCALE)
                    nc.tensor.matmul(pvp[:], lhsT=v_sb[:, ti, :], rhs=ex[:],
                                     start=(ti == 0), stop=(ti == njt - 1))
                    nc.tensor.matmul(rsp[:], lhsT=ones[:], rhs=ex[:],
                                     start=(ti == 0), stop=(ti == njt - 1))
                rec = rope.tile([1, SCH], F32, tag="rec", name="rec")
                nc.vector.reciprocal(out=rec[:], in_=rsp[:])
                bc = rope.tile([P, SCH], F32, tag="bc", name="bc")
                nc.gpsimd.partition_broadcast(bc[:], rec[:], channels=P)
                ot = outp.tile([P, SCH], F32R, tag=f"oT{h}", name=f"oT{h}")
                nc.vector.tensor_mul(out=ot[:], in0=pvp[:], in1=bc[:])
                outT[(h, c)] = ot

        def emit_wo(cpair):
            cs = [2 * cpair, 2 * cpair + 1]
            for et in range(D // P):
                wo_t = stream.tile([P, NQ, P], F32R, tag="wo", name="wo")
                nc.sync.dma_start(wo_t[:], woT3[:, :, et * P:(et + 1) * P])
                ysb = yout.tile([P, 2, SCH], F32, tag="ysb", name="ysb")
                for ci, c in enumerate(cs):
                    yp = ps.tile([P, SCH], F32,
                                 tag=("t4" if et % 2 == 0 else "t5"), name="yp")
                    for h in range(NQ):
                        nc.tensor.matmul(yp[:], lhsT=wo_t[:, h, :],
                                         rhs=outT[(h, c)][:],
                                         start=(h == 0), stop=(h == NQ - 1))
                    nc.any.tensor_copy(out=ysb[:, ci, :], in_=yp[:])
                nc.sync.dma_start(
                    yT_dram[et * P:(et + 1) * P, cpair * 2 * SCH:(cpair + 1) * 2 * SCH],
                    ysb[:])

        for c in range(NCH):
            sl = slice(c * SCH, (c + 1) * SCH)
            # ================= QKV for chunk c =================
            pq = [ps.tile([P, SCH], F32, tag=f"t{h}", name=f"pq{h}")
                  for h in range(NQ)]
            pk = ps.tile([P, SCH], F32, tag="t4", name="pk")
            pv = ps.tile([P, SCH], F32, tag="t5", name="pv")
            for kd in range(NKD):
                xt = stream.tile([P, SCH], F32R, tag="xt", name="xt")
                nc.sync.dma_start(xt[:], xT3[:, kd, sl])
                wkv_t = stream.tile([P, 2 * HD], F32R, tag="wkv", name="wkv")
                nc.sync.dma_start(wkv_t[:], wkvT3[:, kd, :])
                st = kd == 0
                sp = kd == NKD - 1
                for h in range(NQ):
                    nc.tensor.matmul(
                        pq[h][:], lhsT=wq_sb[:, kd, h * HD:(h + 1) * HD],
                        rhs=xt[:], start=st, stop=sp)
                nc.tensor.matmul(pk[:], lhsT=wkv_t[:, 0:HD], rhs=xt[:],
                                 start=st, stop=sp)
                nc.tensor.matmul(pv[:], lhsT=wkv_t[:, HD:2 * HD], rhs=xt[:],
                                 start=st, stop=sp)

            # rope tables for this chunk
            cct = ccs.tile([P, SCH], F32, tag="cc", name="cc")
            nc.sync.dma_start(cct[:], cc_d[:, sl])
            sst = ccs.tile([P, SCH], F32, tag="ss", name="ss")
            nc.sync.dma_start(sst[:], ss_d[:, sl])

            # rope q -> per-chunk qT tiles
            for h in range(NQ):
                qt = qres.tile([P, SCH], F32R, tag=f"qT{h}", name=f"qT{h}")
                rope_evict(pq[h][:], qt[:], cct, sst)
                qT[(h, c)] = qt
            # rope k -> resident kT
            rope_evict(pk[:], kT[:, sl], cct, sst)

            # v: psum -> sbuf, then transpose into v_sb rows; cache_v out merged
            vstage = rope.tile([P, SCH], F32, tag="vstage", name="vstage")
            nc.scalar.copy(out=vstage[:], in_=pv[:])
            for st_i in range(SCH // P):
                jt = c * (SCH // P) + st_i
                pt = ps.tile([P, P], F32, tag="t6", bufs=2, name="tp")
                nc.tensor.transpose(pt[:], vstage[:, st_i * P:(st_i + 1) * P],
                                    ident[:])
                nc.any.tensor_copy(out=v_sb[:, jt, :], in_=pt[:])
            nc.sync.dma_start(cv3[:, 4 * c:4 * (c + 1), :],
                              v_sb[:, 4 * c:4 * (c + 1), :].bitcast(F32))

            # cache_k out: transpose kT chunk, un-permute channels (stride-2)
            ck4 = rope.tile([P, SCH // P, P], F32, tag="ck4", name="ck4")
            for st_i in range(SCH // P):
                jt = c * (SCH // P) + st_i
                pt = ps.tile([P, P], F32, tag="t6", bufs=2, name="tpk")
                nc.tensor.transpose(pt[:], kT[:, jt * P:(jt + 1) * P].bitcast(F32), ident[:])
                nc.vector.tensor_copy(out=ck4[:, st_i, 0:P:2], in_=pt[:, 0:64])
                nc.vector.tensor_copy(out=ck4[:, st_i, 1:P:2], in_=pt[:, 64:128])
            nc.sync.dma_start(ck3[:, 4 * c:4 * (c + 1), :], ck4[:])

            # ================= attention for chunk c =================
            emit_attn(c)
            if c % 2 == 1:
                emit_wo(c // 2)

        # ================= reduce-scatter + final transpose =================
        if single_core:
            nc.sync.dma_start(yT_rs[:], yT_dram[0:D // N_CORES, :])
        else:
            nc.gpsimd.collective_compute(
                "ReduceScatter", mybir.AluOpType.add,
                replica_groups=[list(range(N_CORES))],
                ins=[yT_dram[:]], outs=[yT_rs[:]])

        NET = D // N_CORES // P  # 4 e-tiles of the scattered slice
        for c4 in range(NCH):
            yr4 = yout.tile([P, NET, SCH], F32, tag="yr4", name="yr4")
            nc.sync.dma_start(yr4[:], yrs3[:, :, c4 * SCH:(c4 + 1) * SCH])
            for st_i in range(SCH // P):
                s0 = c4 * SCH + st_i * P
                yst = yout.tile([P, NET * P], F32, tag="yst", name="yst")
                for et in range(NET):
                    pt = ps.tile([P, P], F32, tag="t6", bufs=2, name="tpy")
                    nc.tensor.transpose(pt[:], yr4[:, et, st_i * P:(st_i + 1) * P],
                                        ident[:])
                    nc.any.tensor_copy(out=yst[:, et * P:(et + 1) * P], in_=pt[:])
                nc.sync.dma_start(y_ext[s0:s0 + P, :], yst[:])


def build_nc(single_core=False, reps=1):
    nc = bacc.Bacc("TRN2", target_bir_lowering=False, debug=False,
                   enable_asserts=True,
                   num_devices=1 if single_core else N_CORES)
    io = declare_io(nc)
    with tile.TileContext(nc) as tc:
        for _ in range(reps):
            build_body(nc, tc, io, single_core=single_core)
    nc.compile()
    return nc


def prep_inputs(x, cos_freq, sin_freq, positions, mask, cache_k, cache_v,
                wq, wk, wv, wo):
    """Shard + pre-transpose host-side. Returns in_maps (list of 8 dicts)."""
    x = np.asarray(x, np.float32)
    cos_freq = np.asarray(cos_freq, np.float32)
    sin_freq = np.asarray(sin_freq, np.float32)
    mask = np.asarray(mask, np.float32)
    cache_k = np.asarray(cache_k, np.float32)
    cache_v = np.asarray(cache_v, np.float32)
    wq = np.asarray(wq, np.float32)
    wk = np.asarray(wk, np.float32)
    wv = np.asarray(wv, np.float32)
    wo = np.asarray(wo, np.float32)

    perm = np.concatenate([np.arange(0, HD, 2), np.arange(1, HD, 2)])
    xT = np.ascontiguousarray(x.T)
    cosT = np.ascontiguousarray(cos_freq.T)  # [64, S]
    sinT = np.ascontiguousarray(sin_freq.T)
    cc = np.concatenate([cosT, cosT], axis=0)          # [128, S]
    ss = np.concatenate([-sinT, sinT], axis=0)
    # triangle mask pattern from actual mask values, pre-divided by SCALE
    tri = np.maximum(mask[:P, :P].T, NEG_CLAMP) / SCALE
    tri = np.ascontiguousarray(tri, np.float32)

    wqh = wq.reshape(N_CORES * NQ, HD, D)[:, perm, :]
    wkh = wk.reshape(N_CORES, HD, D)[:, perm, :]
    wvh = wv.reshape(N_CORES, HD, D)

    in_maps = []
    for c in range(N_CORES):
        wq_c = wqh[NQ * c:NQ * (c + 1)].reshape(NQ * HD, D)
        wkv_c = np.concatenate([wkh[c], wvh[c]], axis=0)  # [256, D]
        in_maps.append({
            "xT": xT,
            "wqT": np.ascontiguousarray(wq_c.T),
            "wkvT": np.ascontiguousarray(wkv_c.T),
            "woT": np.ascontiguousarray(wo[:, NQ * HD * c:NQ * HD * (c + 1)].T),
            "cc": cc,
            "ss": ss,
            "tri": tri,
            "cache_k_in": np.ascontiguousarray(cache_k[:, c, :]),
            "cache_v_in": np.ascontiguousarray(cache_v[:, c, :]),
        })
    return in_maps


_NC_CACHE = {}


def _get_nc(key="full", **kw):
    if key not in _NC_CACHE:
        _NC_CACHE[key] = build_nc(**kw)
    return _NC_CACHE[key]


def kernel(**inputs):
    in_maps = prep_inputs(**inputs)
    nc = _get_nc()
    res = run_bass_kernel_spmd(nc, in_maps, core_ids=list(range(N_CORES)))
    outs = res.results
    y = np.concatenate([outs[c]["y"] for c in range(N_CORES)], axis=1)
    ck = np.stack([outs[c]["cache_k"] for c in range(N_CORES)], axis=1)
    cv = np.stack([outs[c]["cache_v"] for c in range(N_CORES)], axis=1)
    return y, ck, cv
